# revision 14
# baseline (speedup 1.0000x reference)
"""Trainium2 kernel for nn_CentralSplitter: stable bucketing of N=1048576
atoms into S=4 species buckets (counting sort), gathering feat_a [N,128] and
feat_b [N,64] f32 into (S, N/S, D) outputs.

Strategy (data-parallel over atoms, 8 cores):
  - Core k owns atoms [k*131072, (k+1)*131072). Host computes, per 32768-atom
    window, the window-local stable counting-sort order (int16 indices).
  - feat_a|feat_b are concatenated on the host into 768B combined rows so one
    gather descriptor moves a whole atom (>=512B keeps SDMA at line rate).
  - Device: 16x dma_gather (8192 rows each) from HBM -> SBUF. The index list
    is PRE-PERMUTED on the host so the gather's interleaved landing
    (dst[i%128, i//128] = src[idx[i]]) leaves each SBUF partition holding 64
    CONSECUTIVE packed output rows -> the store is one fully contiguous 6MB
    DMA per chunk. All shapes static; species-count variability lives only in
    the index values.
  - Host reassembles the species buckets from the per-core packed outputs
    (pure slicing; bucket boundaries come from host-side counts).
"""

import numpy as np

N = 1048576
S = 4
D_A = 128
D_B = 64
D = D_A + D_B            # 192 f32 = 768B combined row
NCORES = 8
LOCAL = N // NCORES      # 131072 atoms per core
WIN = 32768              # int16-addressable gather window
NWIN = LOCAL // WIN      # 4 windows per core
CHUNK = 2048             # rows per dma_gather / per store
NCHUNK = LOCAL // CHUNK  # 32 chunks per core
QPW = WIN // CHUNK       # 8 chunks per window
P = 128
KROWS = CHUNK // P       # 32 rows per partition per chunk
S16 = CHUNK // 16        # 256 idx columns per chunk
NQUEUES = 4              # SWDGE queues (Q7 core pairs) generating descriptors
NBUFS = 10

_nc_cache = None


def _build_nc():
    import concourse.bacc as bacc
    import concourse.mybir as mybir
    from concourse.tile import TileContext

    # 48KB/partition descriptor-ring carveout: ring holds 768 descs per
    # (queue, dir, lane) so ~3 gathers per queue can be in flight; the stock
    # 16KB ring (256 descs) stalls the in-order decode on every 257-desc
    # gather, serializing the Q7 pairs.
    nc = bacc.Bacc("TRN2", target_bir_lowering=False,
                   num_swdge_queues=NQUEUES,
                   dynamic_dma_scratch_size=49152)
    comb = nc.dram_tensor("comb", [LOCAL, D], mybir.dt.float32,
                          kind="ExternalInput")
    # chunk g's int16 index list lives only in the 32-partition group of its
    # queue (g % NQUEUES) — the Q7 core pair for queue q reads partitions
    # [32q, 32q+32) — so 4 chunks share each column block.
    idxhw = nc.dram_tensor("idxhw", [P, (NCHUNK // NQUEUES) * S16],
                           mybir.dt.int16, kind="ExternalInput")
    outc = nc.dram_tensor("outc", [LOCAL, D], mybir.dt.float32,
                          kind="ExternalOutput")

    with TileContext(nc) as tc:
        with (
            tc.tile_pool(name="idxp", bufs=1) as idxp,
            tc.tile_pool(name="datap", bufs=NBUFS) as datap,
        ):
            idx_t = idxp.tile([P, (NCHUNK // NQUEUES) * S16], mybir.dt.int16)
            nc.sync.dma_start(out=idx_t[:], in_=idxhw[:, :])
            for g in range(NCHUNK):
                w = g // QPW
                blk = g // NQUEUES
                data_t = datap.tile([P, KROWS * D], mybir.dt.float32,
                                    tag="data")
                nc.gpsimd.dma_gather(
                    out_ap=data_t[:].rearrange("p (k d) -> p k d", d=D),
                    in_ap=comb[w * WIN:(w + 1) * WIN, :],
                    idxs_ap=idx_t[:, blk * S16:(blk + 1) * S16],
                    num_idxs=CHUNK,
                    num_idxs_reg=CHUNK,
                    elem_size=D,
                    single_packet=False,
                    queue_num=g % NQUEUES,
                )
                weng = nc.sync if g % 2 == 0 else nc.scalar
                weng.dma_start(
                    out=outc[g * CHUNK:(g + 1) * CHUNK, :]
                        .rearrange("(p k) d -> p (k d)", p=P),
                    in_=data_t[:],
                )
    nc.compile()
    return nc


def _get_nc():
    global _nc_cache
    if _nc_cache is None:
        _nc_cache = _build_nc()
    return _nc_cache


def _host_prep(feat_a, feat_b, central_species):
    """Build per-core device inputs + bucket counts for reassembly."""
    comb = np.concatenate([feat_a, feat_b], axis=1)  # [N, 192] f32
    sp = np.ascontiguousarray(central_species)
    counts = np.zeros((NCORES, NWIN, S), dtype=np.int64)
    idxhw = np.empty((NCORES, P, (NCHUNK // NQUEUES) * S16), dtype=np.int16)
    for k in range(NCORES):
        for w in range(NWIN):
            base = k * LOCAL + w * WIN
            spw = sp[base:base + WIN]
            parts = [np.flatnonzero(spw == s) for s in range(S)]
            counts[k, w] = [len(p) for p in parts]
            order = np.concatenate(parts).astype(np.int16)  # [WIN]
            for q in range(QPW):
                g = w * QPW + q
                # landing (p,c) holds packed row p*KROWS+c  =>
                # idxlist[c*128+p] = order[q*CHUNK + p*KROWS + c]
                arr = order[q * CHUNK:(q + 1) * CHUNK].reshape(P, KROWS)
                idxlist = arr.T.ravel()               # [CHUNK], c-major
                wrapped = idxlist.reshape(S16, 16).T  # [16, S16]
                Q, blk = g % NQUEUES, g // NQUEUES
                cols = slice(blk * S16, (blk + 1) * S16)
                idxhw[k, 32 * Q:32 * Q + 16, cols] = wrapped
                idxhw[k, 32 * Q + 16:32 * Q + 32, cols] = wrapped
    in_maps = [
        {"comb": comb[k * LOCAL:(k + 1) * LOCAL], "idxhw": idxhw[k]}
        for k in range(NCORES)
    ]
    return in_maps, counts


def _assemble(results, counts):
    n_per = N // S
    out_a = np.empty((S, n_per, D_A), dtype=np.float32)
    out_b = np.empty((S, n_per, D_B), dtype=np.float32)
    # within a (core, window) block, species s starts at the local prefix sum
    local_off = np.zeros((NCORES, NWIN, S), dtype=np.int64)
    local_off[:, :, 1:] = np.cumsum(counts, axis=2)[:, :, :-1]
    for s in range(S):
        pos = 0
        for k in range(NCORES):
            outc = results[k]["outc"]
            for w in range(NWIN):
                c = int(counts[k, w, s])
                lo = w * WIN + int(local_off[k, w, s])
                blk = outc[lo:lo + c]
                out_a[s, pos:pos + c] = blk[:, :D_A]
                out_b[s, pos:pos + c] = blk[:, D_A:]
                pos += c
        assert pos == n_per
    return out_a, out_b


def kernel(feat_a, feat_b, central_species):
    from concourse.bass_utils import run_bass_kernel_spmd

    nc = _get_nc()
    in_maps, counts = _host_prep(feat_a, feat_b, central_species)
    res = run_bass_kernel_spmd(nc, in_maps, core_ids=list(range(NCORES)))
    return _assemble(res.results, counts)


# revision 15
# speedup vs baseline: 1.1090x; 1.1090x over previous
"""Trainium2 kernel for nn_CentralSplitter: stable bucketing of N=1048576
atoms into S=4 species buckets (counting sort), gathering feat_a [N,128] and
feat_b [N,64] f32 into (S, N/S, D) outputs.

Strategy (data-parallel over atoms, 8 cores):
  - Core k owns atoms [k*131072, (k+1)*131072). Host computes, per 32768-atom
    window, the window-local stable counting-sort order (int16 indices).
  - feat_a|feat_b are concatenated on the host into 768B combined rows so one
    gather descriptor moves a whole atom (>=512B keeps SDMA at line rate).
  - Device: 16x dma_gather (8192 rows each) from HBM -> SBUF. The index list
    is PRE-PERMUTED on the host so the gather's interleaved landing
    (dst[i%128, i//128] = src[idx[i]]) leaves each SBUF partition holding 64
    CONSECUTIVE packed output rows -> the store is one fully contiguous 6MB
    DMA per chunk. All shapes static; species-count variability lives only in
    the index values.
  - Host reassembles the species buckets from the per-core packed outputs
    (pure slicing; bucket boundaries come from host-side counts).
"""

import numpy as np

N = 1048576
S = 4
D_A = 128
D_B = 64
D = D_A + D_B            # 192 f32 = 768B combined row
NCORES = 8
LOCAL = N // NCORES      # 131072 atoms per core
WIN = 32768              # int16-addressable gather window
NWIN = LOCAL // WIN      # 4 windows per core
CHUNK = 2048             # rows per dma_gather / per store
NCHUNK = LOCAL // CHUNK  # 32 chunks per core
QPW = WIN // CHUNK       # 8 chunks per window
P = 128
KROWS = CHUNK // P       # 32 rows per partition per chunk
S16 = CHUNK // 16        # 256 idx columns per chunk
NQUEUES = 4              # SWDGE queues (Q7 core pairs) generating descriptors
NBUFS = 10

_nc_cache = None


def _build_nc():
    import concourse.bacc as bacc
    import concourse.mybir as mybir
    from concourse.tile import TileContext

    # 48KB/partition descriptor-ring carveout: ring holds 768 descs per
    # (queue, dir, lane) so ~3 gathers per queue can be in flight; the stock
    # 16KB ring (256 descs) stalls the in-order decode on every 257-desc
    # gather, serializing the Q7 pairs.
    nc = bacc.Bacc("TRN2", target_bir_lowering=False,
                   num_swdge_queues=NQUEUES,
                   dynamic_dma_scratch_size=49152)
    comb = nc.dram_tensor("comb", [LOCAL, D], mybir.dt.float32,
                          kind="ExternalInput")
    # chunk g's int16 index list lives only in the 32-partition group of its
    # queue (g % NQUEUES) — the Q7 core pair for queue q reads partitions
    # [32q, 32q+32) — so 4 chunks share each column block.
    idxhw = nc.dram_tensor("idxhw", [P, (NCHUNK // NQUEUES) * S16],
                           mybir.dt.int16, kind="ExternalInput")
    outc = nc.dram_tensor("outc", [LOCAL, D], mybir.dt.float32,
                          kind="ExternalOutput")

    with TileContext(nc) as tc:
        with (
            tc.tile_pool(name="idxp", bufs=1) as idxp,
            tc.tile_pool(name="datap", bufs=NBUFS) as datap,
        ):
            idx_t = idxp.tile([P, (NCHUNK // NQUEUES) * S16], mybir.dt.int16)
            nc.sync.dma_start(out=idx_t[:], in_=idxhw[:, :])
            for g in range(NCHUNK):
                w = g // QPW
                blk = g // NQUEUES
                data_t = datap.tile([P, KROWS * D], mybir.dt.float32,
                                    tag="data")
                nc.gpsimd.dma_gather(
                    out_ap=data_t[:].rearrange("p (k d) -> p k d", d=D),
                    in_ap=comb[w * WIN:(w + 1) * WIN, :],
                    idxs_ap=idx_t[:, blk * S16:(blk + 1) * S16],
                    num_idxs=CHUNK,
                    num_idxs_reg=CHUNK,
                    elem_size=D,
                    single_packet=False,
                    queue_num=g % NQUEUES,
                )
                weng = nc.sync if (g % NQUEUES) < 2 else nc.scalar
                weng.dma_start(
                    out=outc[g * CHUNK:(g + 1) * CHUNK, :]
                        .rearrange("(p k) d -> p (k d)", p=P),
                    in_=data_t[:],
                )
    nc.compile()
    return nc


def _get_nc():
    global _nc_cache
    if _nc_cache is None:
        _nc_cache = _build_nc()
    return _nc_cache


def _host_prep(feat_a, feat_b, central_species):
    """Build per-core device inputs + bucket counts for reassembly."""
    comb = np.concatenate([feat_a, feat_b], axis=1)  # [N, 192] f32
    sp = np.ascontiguousarray(central_species)
    counts = np.zeros((NCORES, NWIN, S), dtype=np.int64)
    idxhw = np.empty((NCORES, P, (NCHUNK // NQUEUES) * S16), dtype=np.int16)
    for k in range(NCORES):
        for w in range(NWIN):
            base = k * LOCAL + w * WIN
            spw = sp[base:base + WIN]
            parts = [np.flatnonzero(spw == s) for s in range(S)]
            counts[k, w] = [len(p) for p in parts]
            order = np.concatenate(parts).astype(np.int16)  # [WIN]
            for q in range(QPW):
                g = w * QPW + q
                # landing (p,c) holds packed row p*KROWS+c  =>
                # idxlist[c*128+p] = order[q*CHUNK + p*KROWS + c]
                arr = order[q * CHUNK:(q + 1) * CHUNK].reshape(P, KROWS)
                idxlist = arr.T.ravel()               # [CHUNK], c-major
                wrapped = idxlist.reshape(S16, 16).T  # [16, S16]
                Q, blk = g % NQUEUES, g // NQUEUES
                cols = slice(blk * S16, (blk + 1) * S16)
                idxhw[k, 32 * Q:32 * Q + 16, cols] = wrapped
                idxhw[k, 32 * Q + 16:32 * Q + 32, cols] = wrapped
    in_maps = [
        {"comb": comb[k * LOCAL:(k + 1) * LOCAL], "idxhw": idxhw[k]}
        for k in range(NCORES)
    ]
    return in_maps, counts


def _assemble(results, counts):
    n_per = N // S
    out_a = np.empty((S, n_per, D_A), dtype=np.float32)
    out_b = np.empty((S, n_per, D_B), dtype=np.float32)
    # within a (core, window) block, species s starts at the local prefix sum
    local_off = np.zeros((NCORES, NWIN, S), dtype=np.int64)
    local_off[:, :, 1:] = np.cumsum(counts, axis=2)[:, :, :-1]
    for s in range(S):
        pos = 0
        for k in range(NCORES):
            outc = results[k]["outc"]
            for w in range(NWIN):
                c = int(counts[k, w, s])
                lo = w * WIN + int(local_off[k, w, s])
                blk = outc[lo:lo + c]
                out_a[s, pos:pos + c] = blk[:, :D_A]
                out_b[s, pos:pos + c] = blk[:, D_A:]
                pos += c
        assert pos == n_per
    return out_a, out_b


def kernel(feat_a, feat_b, central_species):
    from concourse.bass_utils import run_bass_kernel_spmd

    nc = _get_nc()
    in_maps, counts = _host_prep(feat_a, feat_b, central_species)
    res = run_bass_kernel_spmd(nc, in_maps, core_ids=list(range(NCORES)))
    return _assemble(res.results, counts)
